# revision 17
# baseline (speedup 1.0000x reference)
"""Trainium2 Bass kernel for a post-LN multi-head-attention block.

Reference computation (B=4, S=2048, D=1024, 16 heads x 64):
    q,k,v = x @ W{q,k,v}.T ; attn = softmax(q k^T/8 + mask) ; o = attn v
    out = LayerNorm(query + (o @ Wo.T)) * gamma + beta

Sharding: 8 cores = 4 batches x 2 query-halves (1024 query rows per core).
Each core computes all 16 heads for its query rows against the full
(mask-compacted) key set of its batch.  No collectives.

Key implementation choices:
  - keys with mask==0 contribute exactly 0 attention weight (additive -1e8
    underflows exp in f32), so the host compacts key/value to the unmasked
    subset, padded to a multiple of 128 (padding biased -1e30 so exp -> 0).
  - all matmuls run in float32r (TF32-like fast fp32 PE path, ~1e-4 rel
    error, 4x the fp32 matmul throughput); softmax and layernorm stay fp32.
  - scores are computed transposed, scoresT[k, q], so softmax's k-reduction
    becomes a matmul reduction: V is augmented with a ones-column and
    attnV produces [out^T ; rowsum] in one PSUM accumulation group.
  - exp/scale/mask fold into one ScalarE activation per tile:
    E = exp(0.125 * scoresT + maskbias[k]).
  - normalization (divide by rowsum) happens after attnV via a K=1
    broadcast matmul of 1/rowsum and an elementwise multiply.
  - K^T/Q^T projections for head-pair j+1 are interleaved into pair j's
    (ScalarE-bound) attention loop so the TensorE never starves.
"""

import numpy as np

import concourse.bacc as bacc
import concourse.tile as tile
import concourse.bass as bass
from concourse import mybir
from concourse.bass_utils import run_bass_kernel_spmd

DMODEL = 1024
NHEAD = 16
HD = 64
B = 4
S = 2048
NCORES = 8
SQ = 1024          # query rows per core
P = 128
F32 = mybir.dt.float32
F32R = mybir.dt.float32r
BF16 = mybir.dt.bfloat16
ET = DMODEL // P   # 8 e-tiles (feature tiles)
DTL = DMODEL // P  # 8 d-tiles (contraction tiles)
NQC = SQ // 512    # 2 query chunks of 512
NEC = DMODEL // 512  # 2 feature chunks of 512
NPAIR = NHEAD // 2   # 8 head pairs; pair j = heads (2j, 2j+1) in e-tile j


def _balanced_chunks(total, maxw=512):
    """Split `total` (a multiple of 128) into ~equal chunks <= maxw,
    each a multiple of 128."""
    nt = total // P
    nch = -(-total // maxw)
    base, rem = divmod(nt, nch)
    out, lo = [], 0
    for i in range(nch):
        w = (base + (1 if i < rem else 0)) * P
        out.append((lo, lo + w))
        lo += w
    return out


def _build(LPAD, do_compile=True, reps=1, phases=5):
    KT = LPAD // P
    KCH = _balanced_chunks(LPAD)
    QCH = _balanced_chunks(SQ)
    # buffer-count knobs: shrink pipeline slack when LPAD-scaled tensors
    # grow, so the kernel still fits SBUF for denser masks
    if LPAD <= 1536:
        BUFS = dict(av_sb=2, wkj=2, wqj=2, kts=2, qts=2)
    else:
        BUFS = dict(av_sb=1, wkj=1, wqj=1, kts=2, qts=2)
    BUFS["vh"] = 512
    nc = bacc.Bacc("TRN2", target_bir_lowering=False, debug=False,
                   num_devices=NCORES)

    qT = nc.declare_dram_parameter("qT", [DMODEL, SQ], BF16, isOutput=False)
    kT = nc.declare_dram_parameter("kT", [DMODEL, LPAD], BF16, isOutput=False)
    vT = nc.declare_dram_parameter("vT", [DMODEL, LPAD], BF16, isOutput=False)
    resid = nc.declare_dram_parameter("resid", [SQ, DMODEL], F32, isOutput=False)
    wqT = nc.declare_dram_parameter("wqT", [DMODEL, DMODEL], BF16, isOutput=False)
    wkT = nc.declare_dram_parameter("wkT", [DMODEL, DMODEL], BF16, isOutput=False)
    wvT = nc.declare_dram_parameter("wvT", [DMODEL, DMODEL], BF16, isOutput=False)
    woT = nc.declare_dram_parameter("woT", [DMODEL, DMODEL], BF16, isOutput=False)
    maskb = nc.declare_dram_parameter("maskb", [P, KT], F32, isOutput=False)
    gamma = nc.declare_dram_parameter("gamma", [DMODEL], F32, isOutput=False)
    beta = nc.declare_dram_parameter("beta", [DMODEL], F32, isOutput=False)
    out = nc.declare_dram_parameter("out", [SQ, DMODEL], F32, isOutput=True)

    def dram3(ap):
        # (o*P, width) DRAM tensor viewed as [p, o, width]
        return ap.rearrange("(o p) w -> p o w", p=P)

    with tile.TileContext(nc) as tc:
        with (
            tc.tile_pool(name="keep", bufs=1) as keep,      # long-lived SBUF
            tc.tile_pool(name="wpool", bufs=1) as wpool,    # weights (phased)
            tc.tile_pool(name="pproj", bufs=2, space="PSUM") as pproj,
            tc.tile_pool(name="pattn", bufs=1, space="PSUM") as pattn,
        ):
            for _rep in range(reps):
                self_body(nc, tc, keep, wpool, pproj, pattn, phases,
                          BUFS, LPAD, KT, KCH, QCH,
                          qT, kT, vT, resid, wqT, wkT, wvT, woT,
                          maskb, gamma, beta, out, dram3)
    if do_compile:
        nc.compile()
    return nc


def self_body(nc, tc, keep, wpool, pproj, pattn, phases, BUFS, LPAD, KT,
              KCH, QCH,
              qT, kT, vT, resid, wqT, wkT, wvT, woT, maskb, gamma, beta,
              out, dram3):
    # ---- long-lived tensors ----
    vaug = keep.tile([P, KT, NHEAD, HD + 1], BF16)   # [k | head | V,1]
    aoT = keep.tile([P, ET, SQ], BF16)               # attn out^T (d' on part)
    maskb_sb = keep.tile([P, KT], F32)
    nc.sync.dma_start(out=maskb_sb, in_=maskb.ap())
    ones_sb = keep.tile([65, 64], F32R)
    nc.vector.memset(ones_sb[64:65, :].bitcast(F32), 1.0)
    for kt in range(KT):
        nc.vector.memset(vaug[:, kt, :, HD:HD + 1], 1.0)
    wo_sb = keep.tile([P, DTL, DMODEL], BF16)        # O-proj weights
    # (prefetched mid-attention; see the j == 1 hook below)

    with (
        tc.tile_pool(name="din", bufs=1) as din,
        tc.tile_pool(name="dpool", bufs=2) as dpool,
    ):
        # resident contraction inputs for the K^T / Q^T projections
        # (DMA'd after phase A's inputs so V-projection starts immediately)
        kin = din.tile([P, DTL, LPAD], BF16)
        qin = din.tile([P, DTL, SQ], BF16)

        vT3 = dram3(vT.ap())
        wvT3 = dram3(wvT.ap())
        kT3 = dram3(kT.ap())
        qT3 = dram3(qT.ap())
        wkT3 = dram3(wkT.ap())
        wqT3 = dram3(wqT.ap())
        wvpool = tc.alloc_tile_pool(name="wvpool", bufs=1)

        VH = BUFS["vh"]
        NVP = DMODEL // VH       # V-projection weight slices
        HPS = VH // HD           # heads per slice

        def vproj_half(ec):
            """Returns trace thunks for one feature-slice of the V
            projection (heads HPS*ec ...).  Weight-slice DMA traced now."""
            wv_h = wvpool.tile([P, DTL, VH], BF16, tag="wv")
            nc.sync.dma_start(out=wv_h,
                              in_=wvT3[:, :, ec * VH:(ec + 1) * VH])
            thunks = []
            for kt in range(KT):
                def vthunk(kt=kt, ec=ec, wv_h=wv_h):
                    vin = dpool.tile([P, DTL, P], BF16, tag="e2", bufs=4)
                    nc.sync.dma_start(out=vin,
                                      in_=vT3[:, :, kt * P:(kt + 1) * P])
                    ps = pproj.tile([P, 512], F32, tag="pp")
                    for dt in range(DTL):
                        nc.tensor.matmul(
                            ps[:, :VH], lhsT=vin[:, dt, :],
                            rhs=wv_h[:, dt, :],
                            start=(dt == 0), stop=(dt == DTL - 1))
                    nc.vector.tensor_copy(
                        out=vaug[:, kt, ec * HPS:(ec + 1) * HPS, 0:HD],
                        in_=ps[:, :VH].rearrange("p (h x) -> p h x", x=HD))
                thunks.append(vthunk)
            return thunks

        def project_pair(j):
            """Trace K^T and Q^T projection for head-pair j (streamed
            per-pair weight column slices).  Returns (kts, qts) tiles plus
            deferred trace thunks (one PSUM group = 8 matmuls + copy each)."""
            wkj = dpool.tile([P, DTL, P], BF16, tag="wkj", bufs=BUFS["wkj"])
            nc.sync.dma_start(out=wkj, in_=wkT3[:, :, j * P:(j + 1) * P])
            wqj = dpool.tile([P, DTL, P], BF16, tag="wqj", bufs=BUFS["wqj"])
            nc.sync.dma_start(out=wqj, in_=wqT3[:, :, j * P:(j + 1) * P])
            kts = dpool.tile([P, LPAD], BF16, tag="kts", bufs=BUFS["kts"])
            qts = dpool.tile([P, SQ], BF16, tag="qts", bufs=BUFS["qts"])
            thunks = []
            for (lo, hi) in KCH:
                def kthunk(lo=lo, hi=hi, kts=kts, wkj=wkj):
                    w = hi - lo
                    ps = pproj.tile([P, 512], F32, tag="pp")
                    for dt in range(DTL):
                        nc.tensor.matmul(
                            ps[:, :w],
                            lhsT=wkj[:, dt, :],
                            rhs=kin[:, dt, lo:hi],
                            start=(dt == 0), stop=(dt == DTL - 1))
                    nc.vector.tensor_copy(out=kts[:, lo:hi], in_=ps[:, :w])
                thunks.append(kthunk)
            for (lo, hi) in QCH:
                def qthunk(lo=lo, hi=hi, qts=qts, wqj=wqj):
                    w = hi - lo
                    ps = pproj.tile([P, 512], F32, tag="pp")
                    for dt in range(DTL):
                        nc.tensor.matmul(
                            ps[:, :w],
                            lhsT=wqj[:, dt, :],
                            rhs=qin[:, dt, lo:hi],
                            start=(dt == 0), stop=(dt == DTL - 1))
                    nc.vector.tensor_copy(out=qts[:, lo:hi], in_=ps[:, :w])
                thunks.append(qthunk)
            return kts, qts, thunks

        if phases < 4:
            # projections only (for phase bisection)
            for ec in range(NVP):
                for t in vproj_half(ec):
                    t()
            wvpool.release()
            for j in range(NPAIR):
                kts, qts, thunks = project_pair(j)
                for t in thunks:
                    t()
                dbg = dpool.tile([P, LPAD + SQ], F32, tag="dbg", bufs=1)
                nc.vector.tensor_copy(out=dbg[:, 0:LPAD], in_=kts)
                nc.vector.tensor_copy(out=dbg[:, LPAD:LPAD + SQ], in_=qts)
                nc.sync.dma_start(out=out.ap()[0:P, 0:LPAD],
                                  in_=dbg[:, 0:LPAD])
                nc.sync.dma_start(out=out.ap()[P:2 * P, 0:SQ],
                                  in_=dbg[:, LPAD:LPAD + SQ])
            return

        # ======== prologue: pair 0's projections + first V half ========
        # DMA order = need order: pair-0 proj weights, kin/qin, wv/vin.
        kts, qts, thunks = project_pair(0)
        for (lo, hi) in KCH:
            nc.sync.dma_start(out=kin[:, :, lo:hi], in_=kT3[:, :, lo:hi])
        for (lo, hi) in QCH:
            nc.sync.dma_start(out=qin[:, :, lo:hi], in_=qT3[:, :, lo:hi])
        vthunks0 = vproj_half(0)
        for t in thunks:          # pair-0 K^T/Q^T projections
            t()
        for t in vthunks0:        # V projection for heads 0..7
            t()

        # ======== phase D: attention; pair j+1's projections and the
        # V-projection halves run as TensorE filler work inside the loop;
        # each (j, qc) epilogue is deferred past the next iteration's
        # first scores so ScalarE never waits at iteration boundaries ====
        pend = [None]
        for j in range(NPAIR):
            if j == 1:
                # prefetch O-proj weights while the DMA queues are quiet
                nc.sync.dma_start(out=wo_sb, in_=dram3(woT.ap()))
            fill = []
            if 0 <= j < NVP - 1:
                fill += vproj_half(j + 1)   # next V-projection slice
            if j + 1 < NPAIR:
                kts_n, qts_n, pf = project_pair(j + 1)
                fill += pf
            else:
                kts_n = qts_n = None

            for qc in range(NQC):
                qsl = slice(qc * 512, (qc + 1) * 512)
                avA = pattn.tile([HD + 1, 512], F32, tag="avA")
                avB = pattn.tile([HD + 1, 512], F32, tag="avB")
                es = []

                def scores(kt):
                    ps = pproj.tile([P, 1024], F32, tag="ps_s", bufs=2)
                    nc.tensor.matmul(
                        ps[:, 0:512],
                        lhsT=kts[0:64, kt * P:(kt + 1) * P],
                        rhs=qts[0:64, qsl], start=True, stop=True)
                    nc.tensor.matmul(
                        ps[:, 512:1024],
                        lhsT=kts[64:128, kt * P:(kt + 1) * P],
                        rhs=qts[64:128, qsl], start=True, stop=True)
                    e2 = dpool.tile([P, 1024], BF16, tag="e2", bufs=4)
                    nc.scalar.activation(
                        out=e2, in_=ps,
                        func=mybir.ActivationFunctionType.Exp,
                        bias=maskb_sb[:, kt:kt + 1], scale=0.125)
                    es.append(e2)

                def attnv(kt, es=es, avA=avA, avB=avB, j=j):
                    e2 = es[kt]
                    nc.tensor.matmul(
                        avA, lhsT=vaug[:, kt, 2 * j, :], rhs=e2[:, 0:512],
                        start=(kt == 0), stop=(kt == KT - 1))
                    nc.tensor.matmul(
                        avB, lhsT=vaug[:, kt, 2 * j + 1, :],
                        rhs=e2[:, 512:1024],
                        start=(kt == 0), stop=(kt == KT - 1))

                # software pipeline: scores(kt) ahead of attnv(kt-1);
                # pair-0 qc0 also interleaves its V-projection groups;
                # previous iteration's epilogue lands after scores(0)
                scores(0)
                if pend[0] is not None:
                    pend[0]()          # prev iteration's attnv tail + epilogue
                    pend[0] = None
                for kt in range(1, KT):
                    scores(kt)
                    attnv(kt - 1)
                    if fill:
                        fill.pop(0)()
                if fill and qc == NQC - 1:
                    while fill:
                        fill.pop(0)()

                def epilogue(avA=avA, avB=avB, j=j, qsl=qsl,
                             attnv=attnv):
                    attnv(KT - 1)      # deferred pipeline tail
                    # drain the PSUM accumulators to SBUF right away so
                    # the banks free up for the next iteration's attnV
                    av_sb = dpool.tile([65, 1024], F32R, tag="av_sb",
                                       bufs=BUFS["av_sb"])
                    nc.vector.tensor_copy(out=av_sb[:, 0:512], in_=avA)
                    nc.vector.tensor_copy(out=av_sb[:, 512:1024], in_=avB)

                    # normalize: aoT = av / rowsum
                    recip_t = dpool.tile([65, 1024], F32R, tag="recip",
                                         bufs=1)
                    with nc.allow_low_precision(
                            reason="f32r recip feeds f32r matmul"):
                        nc.vector.reciprocal(
                            out=recip_t[64:65, :],
                            in_=av_sb[64:65, :].bitcast(F32))
                    # reuse the (drained) attention accumulator banks so
                    # the rb matmuls never block projection-filler PSUM
                    rbA = pattn.tile([64, 512], F32, tag="avA")
                    rbB = pattn.tile([64, 512], F32, tag="avB")
                    nc.tensor.matmul(rbA, lhsT=ones_sb[64:65, :],
                                     rhs=recip_t[64:65, 0:512],
                                     start=True, stop=True)
                    nc.tensor.matmul(rbB, lhsT=ones_sb[64:65, :],
                                     rhs=recip_t[64:65, 512:1024],
                                     start=True, stop=True)
                    # broadcast rows land in recip_t[0:64] (else unused)
                    nc.vector.tensor_copy(out=recip_t[0:64, 0:512],
                                          in_=rbA)
                    nc.vector.tensor_copy(out=recip_t[0:64, 512:1024],
                                          in_=rbB)
                    with nc.allow_low_precision(
                            reason="bf16 attn output feeds bf16 matmul"):
                        nc.vector.tensor_tensor(
                            aoT[0:64, j, qsl],
                            av_sb[0:64, 0:512].bitcast(F32),
                            recip_t[0:64, 0:512].bitcast(F32),
                            mybir.AluOpType.mult)
                        av2 = dpool.tile([64, 512], BF16, tag="av2",
                                         bufs=BUFS["av_sb"])
                        nc.vector.tensor_tensor(
                            av2,
                            av_sb[0:64, 512:1024].bitcast(F32),
                            recip_t[0:64, 512:1024].bitcast(F32),
                            mybir.AluOpType.mult)
                    nc.sync.dma_start(out=aoT[64:128, j, qsl], in_=av2)

                pend[0] = epilogue

            kts, qts = kts_n, qts_n
        if pend[0] is not None:
            pend[0]()
            pend[0] = None
        wvpool.release()

    if phases < 5:
        with tc.tile_pool(name="dump", bufs=1) as dump:
            t = dump.tile([P, 512], F32)
            nc.vector.tensor_copy(out=t, in_=aoT[:, 0, 0:512])
            nc.sync.dma_start(out=out.ap()[0:P, 0:512], in_=t)
        return

    # ======== phase E: output proj + residual + layernorm ========
    with tc.tile_pool(name="epool", bufs=2) as epool:
        gamma_sb = epool.tile([P, DMODEL], F32, tag="gamma", bufs=1)
        nc.gpsimd.dma_start(out=gamma_sb, in_=bass.AP(
            tensor=gamma.ap().tensor, offset=0, ap=[[0, P], [1, DMODEL]]))
        beta_sb = epool.tile([P, DMODEL], F32, tag="beta", bufs=1)
        nc.gpsimd.dma_start(out=beta_sb, in_=bass.AP(
            tensor=beta.ap().tensor, offset=0, ap=[[0, P], [1, DMODEL]]))
        eps_sb = epool.tile([P, 1], F32, tag="eps", bufs=1)
        nc.vector.memset(eps_sb, 1e-5)
        resid3 = dram3(resid.ap())
        out3 = dram3(out.ap())

        for st in range(SQ // P):
            rin = epool.tile([P, DMODEL], F32, tag="rin")
            nc.sync.dma_start(out=rin, in_=resid3[:, st, :])
            x_t = epool.tile([P, DMODEL], F32, tag="x", bufs=3)
            for ec in range(NEC):
                ps = pproj.tile([P, 512], F32, tag="pp")
                for dj in range(DTL):
                    nc.tensor.matmul(
                        ps,
                        lhsT=aoT[:, dj, st * P:(st + 1) * P],
                        rhs=wo_sb[:, dj, ec * 512:(ec + 1) * 512],
                        start=(dj == 0), stop=(dj == DTL - 1))
                nc.vector.tensor_add(
                    out=x_t[:, ec * 512:(ec + 1) * 512],
                    in0=ps, in1=rin[:, ec * 512:(ec + 1) * 512])
            stats = epool.tile([P, 2, 6], F32, tag="stats")
            nc.vector.bn_stats(out=stats[:, 0, :], in_=x_t[:, 0:512])
            nc.vector.bn_stats(out=stats[:, 1, :], in_=x_t[:, 512:1024])
            mv = epool.tile([P, 2], F32, tag="mv")
            nc.vector.bn_aggr(out=mv, in_=stats)
            sd = epool.tile([P, 2], F32, tag="sd")
            nc.scalar.activation(out=sd[:, 0:1], in_=mv[:, 1:2],
                                 func=mybir.ActivationFunctionType.Sqrt,
                                 bias=eps_sb[:, 0:1], scale=1.0)
            nc.vector.reciprocal(out=sd[:, 1:2], in_=sd[:, 0:1])
            nc.vector.tensor_scalar(
                out=x_t, in0=x_t, scalar1=mv[:, 0:1], scalar2=sd[:, 1:2],
                op0=mybir.AluOpType.subtract, op1=mybir.AluOpType.mult)
            nc.gpsimd.tensor_mul(out=x_t, in0=x_t, in1=gamma_sb)
            nc.gpsimd.tensor_add(out=x_t, in0=x_t, in1=beta_sb)
            nc.sync.dma_start(out=out3[:, st, :], in_=x_t)


_cache = {}


def _get_nc(LPAD):
    if LPAD not in _cache:
        _cache[LPAD] = _build(LPAD)
    return _cache[LPAD]


def make_in_maps(query, key, value, mask, Wq, Wk, Wv, Wo, ln_gamma, ln_beta):
    """Host-side sharding: returns (in_maps, LPAD)."""
    import ml_dtypes
    bf16 = ml_dtypes.bfloat16
    f = lambda a: np.ascontiguousarray(np.asarray(a, np.float32))
    h = lambda a: np.ascontiguousarray(np.asarray(a, np.float32).astype(bf16))
    query, key, value = f(query), f(key), f(value)
    mask = np.asarray(mask)
    wqT = h(np.asarray(Wq, np.float32).T)
    wkT = h(np.asarray(Wk, np.float32).T)
    wvT = h(np.asarray(Wv, np.float32).T)
    woT = h(np.asarray(Wo, np.float32).T)
    gamma, beta = f(ln_gamma), f(ln_beta)

    idxs = []
    for b in range(B):
        ix = np.nonzero(mask[b] != 0)[0]
        if len(ix) == 0:
            # all-masked row: the -1e8 bias is common to every key, so the
            # reference softmax reduces to plain softmax over all keys.
            ix = np.arange(S)
        idxs.append(ix)
    Lmax = max(len(ix) for ix in idxs)
    LPAD = max(P, ((Lmax + P - 1) // P) * P)
    KT = LPAD // P

    in_maps = []
    for c in range(NCORES):
        b, g = divmod(c, 2)
        ix = idxs[b]
        L = len(ix)
        kc = np.zeros((LPAD, DMODEL), np.float32)
        kc[:L] = key[b][ix]
        vc = np.zeros((LPAD, DMODEL), np.float32)
        vc[:L] = value[b][ix]
        mb = np.full((LPAD,), -1e30, np.float32)
        mb[:L] = 0.0
        qrows = query[b, g * SQ:(g + 1) * SQ]
        in_maps.append({
            "qT": h(qrows.T),
            "kT": h(kc.T),
            "vT": h(vc.T),
            "resid": np.ascontiguousarray(qrows),
            "wqT": wqT, "wkT": wkT, "wvT": wvT, "woT": woT,
            "maskb": np.ascontiguousarray(mb.reshape(KT, P).T),
            "gamma": gamma, "beta": beta,
        })
    return in_maps, LPAD


def gather_out(results):
    out = np.empty((B, S, DMODEL), np.float32)
    for c in range(NCORES):
        b, g = divmod(c, 2)
        out[b, g * SQ:(g + 1) * SQ] = results[c]["out"]
    return out


def kernel(query, key, value, mask, Wq, Wk, Wv, Wo, ln_gamma, ln_beta):
    in_maps, LPAD = make_in_maps(query, key, value, mask, Wq, Wk, Wv, Wo,
                                 ln_gamma, ln_beta)
    nc = _get_nc(LPAD)
    res = run_bass_kernel_spmd(nc, in_maps, list(range(NCORES)))
    return gather_out(res.results)



# revision 23
# speedup vs baseline: 3.3939x; 3.3939x over previous
"""Trainium2 Bass kernel for a post-LN multi-head-attention block.

Reference computation (B=4, S=2048, D=1024, 16 heads x 64):
    q,k,v = x @ W{q,k,v}.T ; attn = softmax(q k^T/8 + mask) ; o = attn v
    out = LayerNorm(query + (o @ Wo.T)) * gamma + beta

Sharding: 8 cores = 4 batches x 2 query-halves (1024 query rows per core).
Each core computes all 16 heads for its query rows against the full
(mask-compacted) key set of its batch.  No collectives.

Key implementation choices:
  - keys with mask==0 contribute exactly 0 attention weight (additive -1e8
    underflows exp in f32), so the host compacts key/value to the unmasked
    subset, padded to a multiple of 128 (padding biased -1e30 so exp -> 0).
  - all matmuls run in float32r (TF32-like fast fp32 PE path, ~1e-4 rel
    error, 4x the fp32 matmul throughput); softmax and layernorm stay fp32.
  - scores are computed transposed, scoresT[k, q], so softmax's k-reduction
    becomes a matmul reduction: V is augmented with a ones-column and
    attnV produces [out^T ; rowsum] in one PSUM accumulation group.
  - exp/scale/mask fold into one ScalarE activation per tile:
    E = exp(0.125 * scoresT + maskbias[k]).
  - normalization (divide by rowsum) happens after attnV via a K=1
    broadcast matmul of 1/rowsum and an elementwise multiply.
  - K^T/Q^T projections for head-pair j+1 are interleaved into pair j's
    (ScalarE-bound) attention loop so the TensorE never starves.
"""

import numpy as np

import concourse.bacc as bacc
import concourse.tile as tile
import concourse.bass as bass
from concourse import mybir
from concourse.bass_utils import run_bass_kernel_spmd

DMODEL = 1024
NHEAD = 16
HD = 64
B = 4
S = 2048
NCORES = 8
SQ = 1024          # query rows per core
P = 128
F32 = mybir.dt.float32
F32R = mybir.dt.float32r
BF16 = mybir.dt.bfloat16
ET = DMODEL // P   # 8 e-tiles (feature tiles)
DTL = DMODEL // P  # 8 d-tiles (contraction tiles)
NQC = SQ // 512    # 2 query chunks of 512
NEC = DMODEL // 512  # 2 feature chunks of 512
NPAIR = NHEAD // 2   # 8 head pairs; pair j = heads (2j, 2j+1) in e-tile j


def _balanced_chunks(total, maxw=512):
    """Split `total` (a multiple of 128) into ~equal chunks <= maxw,
    each a multiple of 128."""
    nt = total // P
    nch = -(-total // maxw)
    base, rem = divmod(nt, nch)
    out, lo = [], 0
    for i in range(nch):
        w = (base + (1 if i < rem else 0)) * P
        out.append((lo, lo + w))
        lo += w
    return out


def _build(LPAD, do_compile=True, reps=1, phases=5):
    KT = LPAD // P
    KCH = _balanced_chunks(LPAD)
    QCH = _balanced_chunks(SQ)
    # buffer-count knobs: shrink pipeline slack when LPAD-scaled tensors
    # grow, so the kernel still fits SBUF for denser masks
    if LPAD <= 1536:
        BUFS = dict(av_sb=2, wkj=2, wqj=2, kts=2, qts=2)
    else:
        BUFS = dict(av_sb=1, wkj=1, wqj=1, kts=2, qts=2)
    BUFS["vh"] = 512
    nc = bacc.Bacc("TRN2", target_bir_lowering=False, debug=False,
                   num_devices=NCORES)

    qT = nc.declare_dram_parameter("qT", [DMODEL, SQ], BF16, isOutput=False)
    kT = nc.declare_dram_parameter("kT", [DMODEL, LPAD], BF16, isOutput=False)
    vT = nc.declare_dram_parameter("vT", [DMODEL, LPAD], BF16, isOutput=False)
    resid = nc.declare_dram_parameter("resid", [SQ, DMODEL], F32, isOutput=False)
    wqT = nc.declare_dram_parameter("wqT", [DMODEL, DMODEL], BF16, isOutput=False)
    wkT = nc.declare_dram_parameter("wkT", [DMODEL, DMODEL], BF16, isOutput=False)
    wvT = nc.declare_dram_parameter("wvT", [DMODEL, DMODEL], BF16, isOutput=False)
    woT = nc.declare_dram_parameter("woT", [DMODEL, DMODEL], BF16, isOutput=False)
    maskb = nc.declare_dram_parameter("maskb", [P, KT], F32, isOutput=False)
    gamma = nc.declare_dram_parameter("gamma", [DMODEL], F32, isOutput=False)
    beta = nc.declare_dram_parameter("beta", [DMODEL], F32, isOutput=False)
    out = nc.declare_dram_parameter("out", [SQ, DMODEL], F32, isOutput=True)

    def dram3(ap):
        # (o*P, width) DRAM tensor viewed as [p, o, width]
        return ap.rearrange("(o p) w -> p o w", p=P)

    with tile.TileContext(nc) as tc:
        with (
            tc.tile_pool(name="keep", bufs=1) as keep,      # long-lived SBUF
            tc.tile_pool(name="wpool", bufs=1) as wpool,    # weights (phased)
            tc.tile_pool(name="pproj", bufs=2, space="PSUM") as pproj,
            tc.tile_pool(name="pattn", bufs=1, space="PSUM") as pattn,
        ):
            for _rep in range(reps):
                self_body(nc, tc, keep, wpool, pproj, pattn, phases,
                          BUFS, LPAD, KT, KCH, QCH,
                          qT, kT, vT, resid, wqT, wkT, wvT, woT,
                          maskb, gamma, beta, out, dram3)
    if do_compile:
        nc.compile()
    return nc


def self_body(nc, tc, keep, wpool, pproj, pattn, phases, BUFS, LPAD, KT,
              KCH, QCH,
              qT, kT, vT, resid, wqT, wkT, wvT, woT, maskb, gamma, beta,
              out, dram3):
    # ---- long-lived tensors ----
    vaug = keep.tile([P, KT, NHEAD, HD + 1], BF16)   # [k | head | V,1]
    aoT = keep.tile([P, ET, SQ], BF16)               # attn out^T (d' on part)
    maskb_sb = keep.tile([P, KT], F32)
    nc.sync.dma_start(out=maskb_sb, in_=maskb.ap())
    ones_sb = keep.tile([65, 64], F32R)
    nc.vector.memset(ones_sb[64:65, :].bitcast(F32), 1.0)
    for kt in range(KT):
        nc.vector.memset(vaug[:, kt, :, HD:HD + 1], 1.0)
    wo_sb = keep.tile([P, DTL, DMODEL], BF16)        # O-proj weights
    # (prefetched mid-attention; see the j == 1 hook below)

    with (
        tc.tile_pool(name="din", bufs=1) as din,
        tc.tile_pool(name="dpool", bufs=2) as dpool,
    ):
        # resident contraction inputs for the K^T / Q^T projections
        # (DMA'd after phase A's inputs so V-projection starts immediately)
        kin = din.tile([P, DTL, LPAD], BF16)
        qin = din.tile([P, DTL, SQ], BF16)

        vT3 = dram3(vT.ap())
        wvT3 = dram3(wvT.ap())
        kT3 = dram3(kT.ap())
        qT3 = dram3(qT.ap())
        wkT3 = dram3(wkT.ap())
        wqT3 = dram3(wqT.ap())
        wvpool = tc.alloc_tile_pool(name="wvpool", bufs=1)

        VH = BUFS["vh"]
        NVP = DMODEL // VH       # V-projection weight slices
        HPS = VH // HD           # heads per slice

        def vproj_half(ec):
            """Returns trace thunks for one feature-slice of the V
            projection (heads HPS*ec ...).  Weight-slice DMA traced now."""
            wv_h = wvpool.tile([P, DTL, VH], BF16, tag="wv")
            nc.sync.dma_start(out=wv_h,
                              in_=wvT3[:, :, ec * VH:(ec + 1) * VH])
            thunks = []
            for kt in range(KT):
                def vthunk(kt=kt, ec=ec, wv_h=wv_h):
                    vin = dpool.tile([P, DTL, P], BF16, tag="e2", bufs=4)
                    nc.sync.dma_start(out=vin,
                                      in_=vT3[:, :, kt * P:(kt + 1) * P])
                    ps = pproj.tile([P, 512], F32, tag="pp")
                    for dt in range(DTL):
                        nc.tensor.matmul(
                            ps[:, :VH], lhsT=vin[:, dt, :],
                            rhs=wv_h[:, dt, :],
                            start=(dt == 0), stop=(dt == DTL - 1))
                    nc.vector.tensor_copy(
                        out=vaug[:, kt, ec * HPS:(ec + 1) * HPS, 0:HD],
                        in_=ps[:, :VH].rearrange("p (h x) -> p h x", x=HD))
                thunks.append(vthunk)
            return thunks

        def project_pair(j):
            """Trace K^T and Q^T projection for head-pair j (streamed
            per-pair weight column slices).  Returns (kts, qts) tiles plus
            deferred trace thunks (one PSUM group = 8 matmuls + copy each)."""
            wkj = dpool.tile([P, DTL, P], BF16, tag="wkj", bufs=BUFS["wkj"])
            nc.sync.dma_start(out=wkj, in_=wkT3[:, :, j * P:(j + 1) * P])
            wqj = dpool.tile([P, DTL, P], BF16, tag="wqj", bufs=BUFS["wqj"])
            nc.sync.dma_start(out=wqj, in_=wqT3[:, :, j * P:(j + 1) * P])
            kts = dpool.tile([P, LPAD], BF16, tag="kts", bufs=BUFS["kts"])
            qts = dpool.tile([P, SQ], BF16, tag="qts", bufs=BUFS["qts"])
            thunks = []
            for (lo, hi) in KCH:
                def kthunk(lo=lo, hi=hi, kts=kts, wkj=wkj):
                    w = hi - lo
                    ps = pproj.tile([P, 512], F32, tag="pp")
                    for dt in range(DTL):
                        nc.tensor.matmul(
                            ps[:, :w],
                            lhsT=wkj[:, dt, :],
                            rhs=kin[:, dt, lo:hi],
                            start=(dt == 0), stop=(dt == DTL - 1))
                    nc.vector.tensor_copy(out=kts[:, lo:hi], in_=ps[:, :w])
                thunks.append(kthunk)
            for (lo, hi) in QCH:
                def qthunk(lo=lo, hi=hi, qts=qts, wqj=wqj):
                    w = hi - lo
                    ps = pproj.tile([P, 512], F32, tag="pp")
                    for dt in range(DTL):
                        nc.tensor.matmul(
                            ps[:, :w],
                            lhsT=wqj[:, dt, :],
                            rhs=qin[:, dt, lo:hi],
                            start=(dt == 0), stop=(dt == DTL - 1))
                    nc.vector.tensor_copy(out=qts[:, lo:hi], in_=ps[:, :w])
                thunks.append(qthunk)
            return kts, qts, thunks

        if phases < 4:
            # projections only (for phase bisection)
            for (lo, hi) in KCH:
                nc.sync.dma_start(out=kin[:, :, lo:hi], in_=kT3[:, :, lo:hi])
            for (lo, hi) in QCH:
                nc.sync.dma_start(out=qin[:, :, lo:hi], in_=qT3[:, :, lo:hi])
            for ec in range(NVP):
                for t in vproj_half(ec):
                    t()
            wvpool.release()
            for j in range(NPAIR):
                kts, qts, thunks = project_pair(j)
                for t in thunks:
                    t()
                W = min(LPAD, DMODEL)
                dbg = dpool.tile([P, W + SQ], F32, tag="dbg", bufs=1)
                nc.vector.tensor_copy(out=dbg[:, 0:W], in_=kts[:, 0:W])
                nc.vector.tensor_copy(out=dbg[:, W:W + SQ], in_=qts)
                nc.sync.dma_start(out=out.ap()[0:P, 0:W], in_=dbg[:, 0:W])
                nc.sync.dma_start(out=out.ap()[P:2 * P, 0:SQ],
                                  in_=dbg[:, W:W + SQ])
            return

        # ======== prologue: pair 0's projections + first V half ========
        # DMA order = need order: pair-0 proj weights, kin/qin, wv/vin.
        kts, qts, thunks = project_pair(0)
        for (lo, hi) in KCH:
            nc.sync.dma_start(out=kin[:, :, lo:hi], in_=kT3[:, :, lo:hi])
        for (lo, hi) in QCH:
            nc.sync.dma_start(out=qin[:, :, lo:hi], in_=qT3[:, :, lo:hi])
        vthunks0 = vproj_half(0)
        for t in thunks:          # pair-0 K^T/Q^T projections
            t()
        for t in vthunks0:        # V projection for heads 0..7
            t()

        # ======== phase D: attention; pair j+1's projections and the
        # V-projection halves run as TensorE filler work inside the loop;
        # each (j, qc) epilogue is deferred past the next iteration's
        # first scores so ScalarE never waits at iteration boundaries ====
        pend = [None]
        for j in range(NPAIR):
            if j == 1:
                # prefetch O-proj weights while the DMA queues are quiet
                nc.sync.dma_start(out=wo_sb, in_=dram3(woT.ap()))
            fill = []
            if 0 <= j < NVP - 1:
                fill += vproj_half(j + 1)   # next V-projection slice
            if j + 1 < NPAIR:
                kts_n, qts_n, pf = project_pair(j + 1)
                fill += pf
            else:
                kts_n = qts_n = None

            for qc in range(NQC):
                qsl = slice(qc * 512, (qc + 1) * 512)
                avA = pattn.tile([HD + 1, 512], F32, tag="avA")
                avB = pattn.tile([HD + 1, 512], F32, tag="avB")
                es = []

                def scores(kt):
                    ps = pproj.tile([P, 1024], F32, tag="ps_s", bufs=2)
                    nc.tensor.matmul(
                        ps[:, 0:512],
                        lhsT=kts[0:64, kt * P:(kt + 1) * P],
                        rhs=qts[0:64, qsl], start=True, stop=True)
                    nc.tensor.matmul(
                        ps[:, 512:1024],
                        lhsT=kts[64:128, kt * P:(kt + 1) * P],
                        rhs=qts[64:128, qsl], start=True, stop=True)
                    e2 = dpool.tile([P, 1024], BF16, tag="e2", bufs=4)
                    nc.scalar.activation(
                        out=e2, in_=ps,
                        func=mybir.ActivationFunctionType.Exp,
                        bias=maskb_sb[:, kt:kt + 1], scale=0.125)
                    es.append(e2)

                def attnv(kt, es=es, avA=avA, avB=avB, j=j):
                    e2 = es[kt]
                    nc.tensor.matmul(
                        avA, lhsT=vaug[:, kt, 2 * j, :], rhs=e2[:, 0:512],
                        start=(kt == 0), stop=(kt == KT - 1))
                    nc.tensor.matmul(
                        avB, lhsT=vaug[:, kt, 2 * j + 1, :],
                        rhs=e2[:, 512:1024],
                        start=(kt == 0), stop=(kt == KT - 1))

                # software pipeline: scores(kt) ahead of attnv(kt-1);
                # pair-0 qc0 also interleaves its V-projection groups;
                # previous iteration's epilogue lands after scores(0)
                scores(0)
                if pend[0] is not None:
                    pend[0]()          # prev iteration's attnv tail + epilogue
                    pend[0] = None
                for kt in range(1, KT):
                    scores(kt)
                    attnv(kt - 1)
                    if fill:
                        fill.pop(0)()
                if fill and qc == NQC - 1:
                    while fill:
                        fill.pop(0)()

                def epilogue(avA=avA, avB=avB, j=j, qsl=qsl,
                             attnv=attnv):
                    attnv(KT - 1)      # deferred pipeline tail
                    # drain the PSUM accumulators to SBUF right away so
                    # the banks free up for the next iteration's attnV
                    av_sb = dpool.tile([65, 1024], F32R, tag="av_sb",
                                       bufs=BUFS["av_sb"])
                    nc.vector.tensor_copy(out=av_sb[:, 0:512], in_=avA)
                    nc.vector.tensor_copy(out=av_sb[:, 512:1024], in_=avB)

                    # normalize: aoT = av / rowsum
                    recip_t = dpool.tile([65, 1024], F32R, tag="recip",
                                         bufs=1)
                    with nc.allow_low_precision(
                            reason="f32r recip feeds f32r matmul"):
                        nc.vector.reciprocal(
                            out=recip_t[64:65, :],
                            in_=av_sb[64:65, :].bitcast(F32))
                    # rb broadcasts go to the projection-filler PSUM ring
                    # (pp), NOT the attention accumulator banks: reusing
                    # avA/avB here chains the next iteration's attnV behind
                    # this epilogue's multiplies (PSUM WAR serialization)
                    rbA = pproj.tile([64, 512], F32, tag="pp")
                    rbB = pproj.tile([64, 512], F32, tag="pp")
                    nc.tensor.matmul(rbA, lhsT=ones_sb[64:65, :],
                                     rhs=recip_t[64:65, 0:512],
                                     start=True, stop=True)
                    nc.tensor.matmul(rbB, lhsT=ones_sb[64:65, :],
                                     rhs=recip_t[64:65, 512:1024],
                                     start=True, stop=True)
                    # multiplies read the broadcast rows straight from PSUM
                    with nc.allow_low_precision(
                            reason="bf16 attn output feeds bf16 matmul"):
                        nc.vector.tensor_tensor(
                            aoT[0:64, j, qsl],
                            av_sb[0:64, 0:512].bitcast(F32),
                            rbA, mybir.AluOpType.mult)
                        av2 = dpool.tile([64, 512], BF16, tag="av2",
                                         bufs=BUFS["av_sb"])
                        nc.vector.tensor_tensor(
                            av2,
                            av_sb[0:64, 512:1024].bitcast(F32),
                            rbB, mybir.AluOpType.mult)
                    nc.sync.dma_start(out=aoT[64:128, j, qsl], in_=av2)

                pend[0] = epilogue

            kts, qts = kts_n, qts_n
        if pend[0] is not None:
            pend[0]()
            pend[0] = None
        wvpool.release()

    if phases < 5:
        with tc.tile_pool(name="dump", bufs=1) as dump:
            t = dump.tile([P, 512], F32)
            nc.vector.tensor_copy(out=t, in_=aoT[:, 0, 0:512])
            nc.sync.dma_start(out=out.ap()[0:P, 0:512], in_=t)
        return

    # ======== phase E: output proj + residual + layernorm ========
    with tc.tile_pool(name="epool", bufs=2) as epool:
        gamma_sb = epool.tile([P, DMODEL], F32, tag="gamma", bufs=1)
        nc.gpsimd.dma_start(out=gamma_sb, in_=bass.AP(
            tensor=gamma.ap().tensor, offset=0, ap=[[0, P], [1, DMODEL]]))
        beta_sb = epool.tile([P, DMODEL], F32, tag="beta", bufs=1)
        nc.gpsimd.dma_start(out=beta_sb, in_=bass.AP(
            tensor=beta.ap().tensor, offset=0, ap=[[0, P], [1, DMODEL]]))
        eps_sb = epool.tile([P, 1], F32, tag="eps", bufs=1)
        nc.vector.memset(eps_sb, 1e-5)
        resid3 = dram3(resid.ap())
        out3 = dram3(out.ap())

        for st in range(SQ // P):
            rin = epool.tile([P, DMODEL], F32, tag="rin")
            nc.sync.dma_start(out=rin, in_=resid3[:, st, :])
            x_t = epool.tile([P, DMODEL], F32, tag="x", bufs=3)
            for ec in range(NEC):
                ps = pproj.tile([P, 512], F32, tag="pp")
                for dj in range(DTL):
                    nc.tensor.matmul(
                        ps,
                        lhsT=aoT[:, dj, st * P:(st + 1) * P],
                        rhs=wo_sb[:, dj, ec * 512:(ec + 1) * 512],
                        start=(dj == 0), stop=(dj == DTL - 1))
                nc.vector.tensor_add(
                    out=x_t[:, ec * 512:(ec + 1) * 512],
                    in0=ps, in1=rin[:, ec * 512:(ec + 1) * 512])
            stats = epool.tile([P, 2, 6], F32, tag="stats")
            nc.vector.bn_stats(out=stats[:, 0, :], in_=x_t[:, 0:512])
            nc.vector.bn_stats(out=stats[:, 1, :], in_=x_t[:, 512:1024])
            mv = epool.tile([P, 2], F32, tag="mv")
            nc.vector.bn_aggr(out=mv, in_=stats)
            sd = epool.tile([P, 2], F32, tag="sd")
            nc.scalar.activation(out=sd[:, 0:1], in_=mv[:, 1:2],
                                 func=mybir.ActivationFunctionType.Sqrt,
                                 bias=eps_sb[:, 0:1], scale=1.0)
            nc.vector.reciprocal(out=sd[:, 1:2], in_=sd[:, 0:1])
            nc.vector.tensor_scalar(
                out=x_t, in0=x_t, scalar1=mv[:, 0:1], scalar2=sd[:, 1:2],
                op0=mybir.AluOpType.subtract, op1=mybir.AluOpType.mult)
            nc.gpsimd.tensor_mul(out=x_t, in0=x_t, in1=gamma_sb)
            nc.gpsimd.tensor_add(out=x_t, in0=x_t, in1=beta_sb)
            nc.sync.dma_start(out=out3[:, st, :], in_=x_t)


_cache = {}


def _get_nc(LPAD):
    if LPAD not in _cache:
        _cache[LPAD] = _build(LPAD)
    return _cache[LPAD]


def make_in_maps(query, key, value, mask, Wq, Wk, Wv, Wo, ln_gamma, ln_beta):
    """Host-side sharding: returns (in_maps, LPAD)."""
    import ml_dtypes
    bf16 = ml_dtypes.bfloat16
    f = lambda a: np.ascontiguousarray(np.asarray(a, np.float32))
    h = lambda a: np.ascontiguousarray(np.asarray(a, np.float32).astype(bf16))
    query, key, value = f(query), f(key), f(value)
    mask = np.asarray(mask)
    wqT = h(np.asarray(Wq, np.float32).T)
    wkT = h(np.asarray(Wk, np.float32).T)
    wvT = h(np.asarray(Wv, np.float32).T)
    woT = h(np.asarray(Wo, np.float32).T)
    gamma, beta = f(ln_gamma), f(ln_beta)

    idxs = []
    for b in range(B):
        ix = np.nonzero(mask[b] != 0)[0]
        if len(ix) == 0:
            # all-masked row: the -1e8 bias is common to every key, so the
            # reference softmax reduces to plain softmax over all keys.
            ix = np.arange(S)
        idxs.append(ix)
    Lmax = max(len(ix) for ix in idxs)
    LPAD = max(P, ((Lmax + P - 1) // P) * P)
    KT = LPAD // P

    in_maps = []
    for c in range(NCORES):
        b, g = divmod(c, 2)
        ix = idxs[b]
        L = len(ix)
        kc = np.zeros((LPAD, DMODEL), np.float32)
        kc[:L] = key[b][ix]
        vc = np.zeros((LPAD, DMODEL), np.float32)
        vc[:L] = value[b][ix]
        mb = np.full((LPAD,), -1e30, np.float32)
        mb[:L] = 0.0
        qrows = query[b, g * SQ:(g + 1) * SQ]
        in_maps.append({
            "qT": h(qrows.T),
            "kT": h(kc.T),
            "vT": h(vc.T),
            "resid": np.ascontiguousarray(qrows),
            "wqT": wqT, "wkT": wkT, "wvT": wvT, "woT": woT,
            "maskb": np.ascontiguousarray(mb.reshape(KT, P).T),
            "gamma": gamma, "beta": beta,
        })
    return in_maps, LPAD


def gather_out(results):
    out = np.empty((B, S, DMODEL), np.float32)
    for c in range(NCORES):
        b, g = divmod(c, 2)
        out[b, g * SQ:(g + 1) * SQ] = results[c]["out"]
    return out


def kernel(query, key, value, mask, Wq, Wk, Wv, Wo, ln_gamma, ln_beta):
    in_maps, LPAD = make_in_maps(query, key, value, mask, Wq, Wk, Wv, Wo,
                                 ln_gamma, ln_beta)
    nc = _get_nc(LPAD)
    res = run_bass_kernel_spmd(nc, in_maps, list(range(NCORES)))
    return gather_out(res.results)



# revision 26
# speedup vs baseline: 3.4270x; 1.0097x over previous
"""Trainium2 Bass kernel for a post-LN multi-head-attention block.

Reference computation (B=4, S=2048, D=1024, 16 heads x 64):
    q,k,v = x @ W{q,k,v}.T ; attn = softmax(q k^T/8 + mask) ; o = attn v
    out = LayerNorm(query + (o @ Wo.T)) * gamma + beta

Sharding: 8 cores = 4 batches x 2 query-halves (1024 query rows per core).
Each core computes all 16 heads for its query rows against the full
(mask-compacted) key set of its batch.  No collectives.

Key implementation choices:
  - keys with mask==0 contribute exactly 0 attention weight (additive -1e8
    underflows exp in f32), so the host compacts key/value to the unmasked
    subset, padded to a multiple of 128 (padding biased -1e30 so exp -> 0).
  - all matmul operands are bf16 (inputs cast host-side; ~6e-4 final rel
    error vs the 2e-2 gate); PSUM accumulation, softmax and LN stay fp32.
  - scores are computed transposed, scoresT[k, q], so softmax's k-reduction
    becomes a matmul reduction: V is augmented with a ones-column and
    attnV produces [out^T ; rowsum] in one PSUM accumulation group.
  - exp/scale/mask fold into one ScalarE activation per tile:
    E = exp(0.125 * scoresT + maskbias[k]).  The whole kernel is bound by
    this exp stream (18.9M elements at 128 lanes x 1.2 GHz ~= 123us), so
    everything else is arranged to hide beneath it.
  - normalization (divide by rowsum) happens after attnV via a K=1
    broadcast matmul of 1/rowsum and an elementwise multiply.  The
    broadcasts land in the projection PSUM ring (pp), NOT the attention
    accumulator banks: reusing avA/avB there chains the next iteration's
    attnV behind the epilogue's multiplies (PSUM WAR serialization, was
    ~3.2x the total runtime).
  - K^T/Q^T projections for head-pair j+1 are interleaved into pair j's
    attention loop so the TensorE never starves.
  - phase E computes 1/sqrt(var+eps) with a DVE Newton iteration instead
    of ACT Sqrt: Sqrt lives in a different ACT table-set than Exp, and the
    set reload costs ~2.7us each way per rep.
"""

import numpy as np

import concourse.bacc as bacc
import concourse.tile as tile
import concourse.bass as bass
from concourse import mybir
from concourse.bass_utils import run_bass_kernel_spmd

DMODEL = 1024
NHEAD = 16
HD = 64
B = 4
S = 2048
NCORES = 8
SQ = 1024          # query rows per core
P = 128
F32 = mybir.dt.float32
F32R = mybir.dt.float32r
BF16 = mybir.dt.bfloat16
ET = DMODEL // P   # 8 e-tiles (feature tiles)
DTL = DMODEL // P  # 8 d-tiles (contraction tiles)
NQC = SQ // 512    # 2 query chunks of 512
NEC = DMODEL // 512  # 2 feature chunks of 512
NPAIR = NHEAD // 2   # 8 head pairs; pair j = heads (2j, 2j+1) in e-tile j


def _balanced_chunks(total, maxw=512):
    """Split `total` (a multiple of 128) into ~equal chunks <= maxw,
    each a multiple of 128."""
    nt = total // P
    nch = -(-total // maxw)
    base, rem = divmod(nt, nch)
    out, lo = [], 0
    for i in range(nch):
        w = (base + (1 if i < rem else 0)) * P
        out.append((lo, lo + w))
        lo += w
    return out


def _build(LPAD, do_compile=True, reps=1, phases=5):
    KT = LPAD // P
    KCH = _balanced_chunks(LPAD)
    QCH = _balanced_chunks(SQ)
    # buffer-count knobs: shrink pipeline slack when LPAD-scaled tensors
    # grow, so the kernel still fits SBUF for denser masks
    if LPAD <= 1536:
        BUFS = dict(av_sb=2, wkj=2, wqj=2, kts=2, qts=2)
    else:
        BUFS = dict(av_sb=1, wkj=1, wqj=1, kts=2, qts=2)
    BUFS["vh"] = 512
    nc = bacc.Bacc("TRN2", target_bir_lowering=False, debug=False,
                   num_devices=NCORES)

    qT = nc.declare_dram_parameter("qT", [DMODEL, SQ], BF16, isOutput=False)
    kT = nc.declare_dram_parameter("kT", [DMODEL, LPAD], BF16, isOutput=False)
    vT = nc.declare_dram_parameter("vT", [DMODEL, LPAD], BF16, isOutput=False)
    resid = nc.declare_dram_parameter("resid", [SQ, DMODEL], F32, isOutput=False)
    wqT = nc.declare_dram_parameter("wqT", [DMODEL, DMODEL], BF16, isOutput=False)
    wkT = nc.declare_dram_parameter("wkT", [DMODEL, DMODEL], BF16, isOutput=False)
    wvT = nc.declare_dram_parameter("wvT", [DMODEL, DMODEL], BF16, isOutput=False)
    woT = nc.declare_dram_parameter("woT", [DMODEL, DMODEL], BF16, isOutput=False)
    maskb = nc.declare_dram_parameter("maskb", [P, KT], F32, isOutput=False)
    gamma = nc.declare_dram_parameter("gamma", [DMODEL], F32, isOutput=False)
    beta = nc.declare_dram_parameter("beta", [DMODEL], F32, isOutput=False)
    out = nc.declare_dram_parameter("out", [SQ, DMODEL], F32, isOutput=True)

    def dram3(ap):
        # (o*P, width) DRAM tensor viewed as [p, o, width]
        return ap.rearrange("(o p) w -> p o w", p=P)

    with tile.TileContext(nc) as tc:
        with (
            tc.tile_pool(name="keep", bufs=1) as keep,      # long-lived SBUF
            tc.tile_pool(name="wpool", bufs=1) as wpool,    # weights (phased)
            tc.tile_pool(name="pproj", bufs=2, space="PSUM") as pproj,
            tc.tile_pool(name="pattn", bufs=1, space="PSUM") as pattn,
        ):
            for _rep in range(reps):
                self_body(nc, tc, keep, wpool, pproj, pattn, phases,
                          BUFS, LPAD, KT, KCH, QCH,
                          qT, kT, vT, resid, wqT, wkT, wvT, woT,
                          maskb, gamma, beta, out, dram3)
    if do_compile:
        nc.compile()
    return nc


def self_body(nc, tc, keep, wpool, pproj, pattn, phases, BUFS, LPAD, KT,
              KCH, QCH,
              qT, kT, vT, resid, wqT, wkT, wvT, woT, maskb, gamma, beta,
              out, dram3):
    # ---- long-lived tensors ----
    vaug = keep.tile([P, KT, NHEAD, HD + 1], BF16)   # [k | head | V,1]
    aoT = keep.tile([P, ET, SQ], BF16)               # attn out^T (d' on part)
    maskb_sb = keep.tile([P, KT], F32)
    nc.sync.dma_start(out=maskb_sb, in_=maskb.ap())
    ones_sb = keep.tile([65, 64], F32R)
    nc.vector.memset(ones_sb[64:65, :].bitcast(F32), 1.0)
    for kt in range(KT):
        nc.vector.memset(vaug[:, kt, :, HD:HD + 1], 1.0)
    wo_sb = keep.tile([P, DTL, DMODEL], BF16)        # O-proj weights
    # (prefetched mid-attention; see the j == 1 hook below)

    with (
        tc.tile_pool(name="din", bufs=1) as din,
        tc.tile_pool(name="dpool", bufs=2) as dpool,
    ):
        # resident contraction inputs for the K^T / Q^T projections
        # (DMA'd after phase A's inputs so V-projection starts immediately)
        kin = din.tile([P, DTL, LPAD], BF16)
        qin = din.tile([P, DTL, SQ], BF16)

        vT3 = dram3(vT.ap())
        wvT3 = dram3(wvT.ap())
        kT3 = dram3(kT.ap())
        qT3 = dram3(qT.ap())
        wkT3 = dram3(wkT.ap())
        wqT3 = dram3(wqT.ap())
        wvpool = tc.alloc_tile_pool(name="wvpool", bufs=1)

        VH = BUFS["vh"]
        NVP = DMODEL // VH       # V-projection weight slices
        HPS = VH // HD           # heads per slice

        def vproj_half(ec):
            """Returns trace thunks for one feature-slice of the V
            projection (heads HPS*ec ...).  Weight-slice DMA traced now."""
            wv_h = wvpool.tile([P, DTL, VH], BF16, tag="wv")
            nc.sync.dma_start(out=wv_h,
                              in_=wvT3[:, :, ec * VH:(ec + 1) * VH])
            thunks = []
            for kt in range(KT):
                def vthunk(kt=kt, ec=ec, wv_h=wv_h):
                    vin = dpool.tile([P, DTL, P], BF16, tag="e2", bufs=4)
                    nc.sync.dma_start(out=vin,
                                      in_=vT3[:, :, kt * P:(kt + 1) * P])
                    ps = pproj.tile([P, 512], F32, tag="pp")
                    for dt in range(DTL):
                        nc.tensor.matmul(
                            ps[:, :VH], lhsT=vin[:, dt, :],
                            rhs=wv_h[:, dt, :],
                            start=(dt == 0), stop=(dt == DTL - 1))
                    nc.vector.tensor_copy(
                        out=vaug[:, kt, ec * HPS:(ec + 1) * HPS, 0:HD],
                        in_=ps[:, :VH].rearrange("p (h x) -> p h x", x=HD))
                thunks.append(vthunk)
            return thunks

        def project_pair(j):
            """Trace K^T and Q^T projection for head-pair j (streamed
            per-pair weight column slices).  Returns (kts, qts) tiles plus
            deferred trace thunks (one PSUM group = 8 matmuls + copy each)."""
            wkj = dpool.tile([P, DTL, P], BF16, tag="wkj", bufs=BUFS["wkj"])
            nc.sync.dma_start(out=wkj, in_=wkT3[:, :, j * P:(j + 1) * P])
            wqj = dpool.tile([P, DTL, P], BF16, tag="wqj", bufs=BUFS["wqj"])
            nc.sync.dma_start(out=wqj, in_=wqT3[:, :, j * P:(j + 1) * P])
            kts = dpool.tile([P, LPAD], BF16, tag="kts", bufs=BUFS["kts"])
            qts = dpool.tile([P, SQ], BF16, tag="qts", bufs=BUFS["qts"])
            thunks = []
            for (lo, hi) in KCH:
                def kthunk(lo=lo, hi=hi, kts=kts, wkj=wkj):
                    w = hi - lo
                    ps = pproj.tile([P, 512], F32, tag="pp")
                    for dt in range(DTL):
                        nc.tensor.matmul(
                            ps[:, :w],
                            lhsT=wkj[:, dt, :],
                            rhs=kin[:, dt, lo:hi],
                            start=(dt == 0), stop=(dt == DTL - 1))
                    nc.vector.tensor_copy(out=kts[:, lo:hi], in_=ps[:, :w])
                thunks.append(kthunk)
            for (lo, hi) in QCH:
                def qthunk(lo=lo, hi=hi, qts=qts, wqj=wqj):
                    w = hi - lo
                    ps = pproj.tile([P, 512], F32, tag="pp")
                    for dt in range(DTL):
                        nc.tensor.matmul(
                            ps[:, :w],
                            lhsT=wqj[:, dt, :],
                            rhs=qin[:, dt, lo:hi],
                            start=(dt == 0), stop=(dt == DTL - 1))
                    nc.vector.tensor_copy(out=qts[:, lo:hi], in_=ps[:, :w])
                thunks.append(qthunk)
            return kts, qts, thunks

        if phases < 4:
            # projections only (for phase bisection)
            for (lo, hi) in KCH:
                nc.sync.dma_start(out=kin[:, :, lo:hi], in_=kT3[:, :, lo:hi])
            for (lo, hi) in QCH:
                nc.sync.dma_start(out=qin[:, :, lo:hi], in_=qT3[:, :, lo:hi])
            for ec in range(NVP):
                for t in vproj_half(ec):
                    t()
            wvpool.release()
            for j in range(NPAIR):
                kts, qts, thunks = project_pair(j)
                for t in thunks:
                    t()
                W = min(LPAD, DMODEL)
                dbg = dpool.tile([P, W + SQ], F32, tag="dbg", bufs=1)
                nc.vector.tensor_copy(out=dbg[:, 0:W], in_=kts[:, 0:W])
                nc.vector.tensor_copy(out=dbg[:, W:W + SQ], in_=qts)
                nc.sync.dma_start(out=out.ap()[0:P, 0:W], in_=dbg[:, 0:W])
                nc.sync.dma_start(out=out.ap()[P:2 * P, 0:SQ],
                                  in_=dbg[:, W:W + SQ])
            return

        # ======== prologue: pair 0's projections + first V half ========
        # DMA order = need order: pair-0 proj weights, kin/qin, wv/vin.
        kts, qts, thunks = project_pair(0)
        for (lo, hi) in KCH:
            nc.sync.dma_start(out=kin[:, :, lo:hi], in_=kT3[:, :, lo:hi])
        for (lo, hi) in QCH:
            nc.sync.dma_start(out=qin[:, :, lo:hi], in_=qT3[:, :, lo:hi])
        vthunks0 = vproj_half(0)
        for t in thunks:          # pair-0 K^T/Q^T projections
            t()
        for t in vthunks0:        # V projection for heads 0..7
            t()

        # ======== phase D: attention; pair j+1's projections and the
        # V-projection halves run as TensorE filler work inside the loop;
        # each (j, qc) epilogue is deferred past the next iteration's
        # first scores so ScalarE never waits at iteration boundaries ====
        pend = [None]
        for j in range(NPAIR):
            if j == 1:
                # prefetch O-proj weights while the DMA queues are quiet
                nc.sync.dma_start(out=wo_sb, in_=dram3(woT.ap()))
            fill = []
            if 0 <= j < NVP - 1:
                fill += vproj_half(j + 1)   # next V-projection slice
            if j + 1 < NPAIR:
                kts_n, qts_n, pf = project_pair(j + 1)
                fill += pf
            else:
                kts_n = qts_n = None

            for qc in range(NQC):
                qsl = slice(qc * 512, (qc + 1) * 512)
                avA = pattn.tile([HD + 1, 512], F32, tag="avA")
                avB = pattn.tile([HD + 1, 512], F32, tag="avB")
                es = []

                def scores(kt):
                    ps = pproj.tile([P, 1024], F32, tag="ps_s", bufs=2)
                    nc.tensor.matmul(
                        ps[:, 0:512],
                        lhsT=kts[0:64, kt * P:(kt + 1) * P],
                        rhs=qts[0:64, qsl], start=True, stop=True)
                    nc.tensor.matmul(
                        ps[:, 512:1024],
                        lhsT=kts[64:128, kt * P:(kt + 1) * P],
                        rhs=qts[64:128, qsl], start=True, stop=True)
                    e2 = dpool.tile([P, 1024], BF16, tag="e2", bufs=4)
                    nc.scalar.activation(
                        out=e2, in_=ps,
                        func=mybir.ActivationFunctionType.Exp,
                        bias=maskb_sb[:, kt:kt + 1], scale=0.125)
                    es.append(e2)

                def attnv(kt, es=es, avA=avA, avB=avB, j=j):
                    e2 = es[kt]
                    nc.tensor.matmul(
                        avA, lhsT=vaug[:, kt, 2 * j, :], rhs=e2[:, 0:512],
                        start=(kt == 0), stop=(kt == KT - 1))
                    nc.tensor.matmul(
                        avB, lhsT=vaug[:, kt, 2 * j + 1, :],
                        rhs=e2[:, 512:1024],
                        start=(kt == 0), stop=(kt == KT - 1))

                # software pipeline: scores(kt) ahead of attnv(kt-1);
                # pair-0 qc0 also interleaves its V-projection groups;
                # previous iteration's epilogue lands after scores(0)
                scores(0)
                if pend[0] is not None:
                    pend[0]()          # prev iteration's attnv tail + epilogue
                    pend[0] = None
                for kt in range(1, KT):
                    scores(kt)
                    attnv(kt - 1)
                    if fill:
                        fill.pop(0)()
                if fill and qc == NQC - 1:
                    while fill:
                        fill.pop(0)()

                def epilogue(avA=avA, avB=avB, j=j, qsl=qsl,
                             attnv=attnv):
                    attnv(KT - 1)      # deferred pipeline tail
                    # drain the PSUM accumulators to SBUF right away so
                    # the banks free up for the next iteration's attnV
                    av_sb = dpool.tile([65, 1024], F32R, tag="av_sb",
                                       bufs=BUFS["av_sb"])
                    nc.vector.tensor_copy(out=av_sb[:, 0:512], in_=avA)
                    nc.vector.tensor_copy(out=av_sb[:, 512:1024], in_=avB)

                    # normalize: aoT = av / rowsum
                    recip_t = dpool.tile([65, 1024], F32R, tag="recip",
                                         bufs=1)
                    with nc.allow_low_precision(
                            reason="f32r recip feeds f32r matmul"):
                        nc.vector.reciprocal(
                            out=recip_t[64:65, :],
                            in_=av_sb[64:65, :].bitcast(F32))
                    # rb broadcasts go to the projection-filler PSUM ring
                    # (pp), NOT the attention accumulator banks: reusing
                    # avA/avB here chains the next iteration's attnV behind
                    # this epilogue's multiplies (PSUM WAR serialization)
                    rbA = pproj.tile([64, 512], F32, tag="pp")
                    rbB = pproj.tile([64, 512], F32, tag="pp")
                    nc.tensor.matmul(rbA, lhsT=ones_sb[64:65, :],
                                     rhs=recip_t[64:65, 0:512],
                                     start=True, stop=True)
                    nc.tensor.matmul(rbB, lhsT=ones_sb[64:65, :],
                                     rhs=recip_t[64:65, 512:1024],
                                     start=True, stop=True)
                    # multiplies read the broadcast rows straight from PSUM
                    with nc.allow_low_precision(
                            reason="bf16 attn output feeds bf16 matmul"):
                        nc.vector.tensor_tensor(
                            aoT[0:64, j, qsl],
                            av_sb[0:64, 0:512].bitcast(F32),
                            rbA, mybir.AluOpType.mult)
                        av2 = dpool.tile([64, 512], BF16, tag="av2",
                                         bufs=BUFS["av_sb"])
                        nc.vector.tensor_tensor(
                            av2,
                            av_sb[0:64, 512:1024].bitcast(F32),
                            rbB, mybir.AluOpType.mult)
                    nc.sync.dma_start(out=aoT[64:128, j, qsl], in_=av2)

                pend[0] = epilogue

            kts, qts = kts_n, qts_n
        if pend[0] is not None:
            pend[0]()
            pend[0] = None
        wvpool.release()

    if phases < 5:
        with tc.tile_pool(name="dump", bufs=1) as dump:
            t = dump.tile([P, 512], F32)
            nc.vector.tensor_copy(out=t, in_=aoT[:, 0, 0:512])
            nc.sync.dma_start(out=out.ap()[0:P, 0:512], in_=t)
        return

    # ======== phase E: output proj + residual + layernorm ========
    with tc.tile_pool(name="epool", bufs=2) as epool:
        gamma_sb = epool.tile([P, DMODEL], F32, tag="gamma", bufs=1)
        nc.gpsimd.dma_start(out=gamma_sb, in_=bass.AP(
            tensor=gamma.ap().tensor, offset=0, ap=[[0, P], [1, DMODEL]]))
        beta_sb = epool.tile([P, DMODEL], F32, tag="beta", bufs=1)
        nc.gpsimd.dma_start(out=beta_sb, in_=bass.AP(
            tensor=beta.ap().tensor, offset=0, ap=[[0, P], [1, DMODEL]]))
        resid3 = dram3(resid.ap())
        out3 = dram3(out.ap())

        NST = SQ // P
        mvall = epool.tile([P, NST, 2], F32, tag="mvall", bufs=1)
        xts = []
        for st in range(NST):
            rin = epool.tile([P, DMODEL], F32, tag="rin")
            nc.sync.dma_start(out=rin, in_=resid3[:, st, :])
            x_t = epool.tile([P, DMODEL], F32, tag="x", bufs=NST)
            for ec in range(NEC):
                ps = pproj.tile([P, 512], F32, tag="pp")
                for dj in range(DTL):
                    nc.tensor.matmul(
                        ps,
                        lhsT=aoT[:, dj, st * P:(st + 1) * P],
                        rhs=wo_sb[:, dj, ec * 512:(ec + 1) * 512],
                        start=(dj == 0), stop=(dj == DTL - 1))
                nc.vector.tensor_add(
                    out=x_t[:, ec * 512:(ec + 1) * 512],
                    in0=ps, in1=rin[:, ec * 512:(ec + 1) * 512])
            stats = epool.tile([P, 2, 6], F32, tag="stats")
            nc.vector.bn_stats(out=stats[:, 0, :], in_=x_t[:, 0:512])
            nc.vector.bn_stats(out=stats[:, 1, :], in_=x_t[:, 512:1024])
            nc.vector.bn_aggr(out=mvall[:, st, :], in_=stats)
            xts.append(x_t)

        # rstd = 1/sqrt(var+eps) for all row-tiles at once, entirely on
        # DVE (Quake seed + 2 Newton steps): an ACT Sqrt here would force
        # a table-set reload (~2.7us each way) right after the attention
        # loop's Exp stream
        I32 = mybir.dt.int32
        v8 = epool.tile([P, NST], F32, tag="v8", bufs=1)
        nc.vector.tensor_scalar_add(out=v8, in0=mvall[:, :, 1],
                                    scalar1=1e-5)
        y8 = epool.tile([P, NST], F32, tag="y8", bufs=1)
        t8 = epool.tile([P, NST], F32, tag="t8", bufs=1)
        with nc.allow_low_precision(reason="newton rsqrt seed bit-trick"):
            nc.vector.tensor_scalar(
                out=y8.bitcast(I32), in0=v8.bitcast(I32), scalar1=1,
                scalar2=None, op0=mybir.AluOpType.logical_shift_right)
            nc.vector.tensor_scalar(
                out=y8.bitcast(I32), in0=y8.bitcast(I32), scalar1=-1,
                scalar2=0x5f3759df, op0=mybir.AluOpType.mult,
                op1=mybir.AluOpType.add)
            for _ in range(2):
                nc.vector.tensor_tensor(t8, y8, y8, mybir.AluOpType.mult)
                nc.vector.tensor_tensor(t8, t8, v8, mybir.AluOpType.mult)
                nc.vector.tensor_scalar(
                    out=t8, in0=t8, scalar1=-0.5, scalar2=1.5,
                    op0=mybir.AluOpType.mult, op1=mybir.AluOpType.add)
                nc.vector.tensor_tensor(y8, y8, t8, mybir.AluOpType.mult)

        for st in range(NST):
            x_t = xts[st]
            nc.vector.tensor_scalar(
                out=x_t, in0=x_t, scalar1=mvall[:, st, 0:1],
                scalar2=y8[:, st:st + 1],
                op0=mybir.AluOpType.subtract, op1=mybir.AluOpType.mult)
            nc.gpsimd.tensor_mul(out=x_t, in0=x_t, in1=gamma_sb)
            nc.gpsimd.tensor_add(out=x_t, in0=x_t, in1=beta_sb)
            nc.sync.dma_start(out=out3[:, st, :], in_=x_t)


_cache = {}


def _get_nc(LPAD):
    if LPAD not in _cache:
        _cache[LPAD] = _build(LPAD)
    return _cache[LPAD]


def make_in_maps(query, key, value, mask, Wq, Wk, Wv, Wo, ln_gamma, ln_beta):
    """Host-side sharding: returns (in_maps, LPAD)."""
    import ml_dtypes
    bf16 = ml_dtypes.bfloat16
    f = lambda a: np.ascontiguousarray(np.asarray(a, np.float32))
    h = lambda a: np.ascontiguousarray(np.asarray(a, np.float32).astype(bf16))
    query, key, value = f(query), f(key), f(value)
    mask = np.asarray(mask)
    wqT = h(np.asarray(Wq, np.float32).T)
    wkT = h(np.asarray(Wk, np.float32).T)
    wvT = h(np.asarray(Wv, np.float32).T)
    woT = h(np.asarray(Wo, np.float32).T)
    gamma, beta = f(ln_gamma), f(ln_beta)

    idxs = []
    for b in range(B):
        ix = np.nonzero(mask[b] != 0)[0]
        if len(ix) == 0:
            # all-masked row: the -1e8 bias is common to every key, so the
            # reference softmax reduces to plain softmax over all keys.
            ix = np.arange(S)
        idxs.append(ix)
    Lmax = max(len(ix) for ix in idxs)
    LPAD = max(P, ((Lmax + P - 1) // P) * P)
    KT = LPAD // P

    in_maps = []
    for c in range(NCORES):
        b, g = divmod(c, 2)
        ix = idxs[b]
        L = len(ix)
        kc = np.zeros((LPAD, DMODEL), np.float32)
        kc[:L] = key[b][ix]
        vc = np.zeros((LPAD, DMODEL), np.float32)
        vc[:L] = value[b][ix]
        mb = np.full((LPAD,), -1e30, np.float32)
        mb[:L] = 0.0
        qrows = query[b, g * SQ:(g + 1) * SQ]
        in_maps.append({
            "qT": h(qrows.T),
            "kT": h(kc.T),
            "vT": h(vc.T),
            "resid": np.ascontiguousarray(qrows),
            "wqT": wqT, "wkT": wkT, "wvT": wvT, "woT": woT,
            "maskb": np.ascontiguousarray(mb.reshape(KT, P).T),
            "gamma": gamma, "beta": beta,
        })
    return in_maps, LPAD


def gather_out(results):
    out = np.empty((B, S, DMODEL), np.float32)
    for c in range(NCORES):
        b, g = divmod(c, 2)
        out[b, g * SQ:(g + 1) * SQ] = results[c]["out"]
    return out


def kernel(query, key, value, mask, Wq, Wk, Wv, Wo, ln_gamma, ln_beta):
    in_maps, LPAD = make_in_maps(query, key, value, mask, Wq, Wk, Wv, Wo,
                                 ln_gamma, ln_beta)
    nc = _get_nc(LPAD)
    res = run_bass_kernel_spmd(nc, in_maps, list(range(NCORES)))
    return gather_out(res.results)

